# revision 8
# baseline (speedup 1.0000x reference)
"""Causal self-attention on 8 TRN2 NeuronCores — pipelined restructure.

Sharding: pure data-parallel on batch (B=8 -> one batch element per core,
no collectives). Each core computes its full [T, C] output slice.

Differences vs the phase-serial baseline:
  * Software-pipelined emission: qk/V/proj GEMM groups are woven between
    attention units so the ACT-bound exp work overlaps the PE-bound GEMMs.
  * DMA order: small consts + xT(t0) + first wqk chunks land first; PE
    starts ~4us in instead of ~21us.
  * S PSUM tiles hold both heads of one k-tile ([P, 2, QC]); one exp
    instruction covers both heads, exact causal column clipping.
  * Causal mask applied only to the 128-wide diagonal blocks, on the Pool
    (gpsimd) engine with a broadcast tril tile.
  * qkT copyback runs on ACT (Copy), V/proj copybacks on DVE: balances
    PSUM->SBUF traffic across both PSUM-capable engines.
  * Softmax epilogue (1/s broadcast along partitions) via gpsimd
    partition_broadcast (USE_PB=True) or a PE ones-matmul into the free
    partitions 64:128 of the pav bank (USE_PB=False); emission of the
    epilogue is deferred one unit so PE never stalls on the reciprocal.
"""

import os
import sys
from contextlib import ExitStack

import numpy as np

try:
    import ml_dtypes
except ImportError:  # pragma: no cover
    sys.path.insert(0, "/opt/trn_rl_repo")
    import ml_dtypes

BF16 = ml_dtypes.bfloat16

B, T, C = 8, 1024, 1024
H, HD = 16, 64
N_CORES = 8

USE_PB = True  # gpsimd partition_broadcast for 1/s (else PE ones-matmul)

TRACE = False
LAST_EXEC_NS = None
LAST_RESULTS = None

_NC_CACHE = {}


def _build_nc(Tp, Cp, Hp, HDp, reps=1, with_bias_qk=False, with_bias_v=True,
              with_bias_p=True):
    import concourse.bass as bass
    import concourse.tile as tile
    from concourse import bacc, mybir

    bf = mybir.dt.bfloat16
    f32 = mybir.dt.float32
    fp8e3 = mybir.dt.float8e3
    AF = mybir.ActivationFunctionType

    P = 128
    CT = Cp // P            # contraction tiles
    TT = Tp // P            # t-tiles
    QC = min(512, Tp)       # q-chunk width
    NQ = Tp // QC           # q-chunks
    TCH = min(512, Tp)      # t-chunk width for qkT
    TJ = Tp // TCH
    DIAG = QC // P          # diagonal k-tiles per q-chunk
    M2C = 2 * Cp // P       # qk m-chunks
    VJ = Cp // QC           # v/proj column chunks
    NU = Hp // 2            # head-pair units

    nc = bacc.Bacc("TRN2", target_bir_lowering=False, debug=False)

    xT_d = nc.declare_dram_parameter("xT", [Cp, Tp], bf, isOutput=False)
    wqk_d = nc.declare_dram_parameter("w_qk", [Cp, 2 * Cp], bf, isOutput=False)
    wv_d = nc.declare_dram_parameter("w_v", [Cp, Cp], bf, isOutput=False)
    wp_d = nc.declare_dram_parameter("w_proj", [Cp, Cp], bf, isOutput=False)
    bqk_d = nc.declare_dram_parameter("b_qk", [M2C, P], f32, isOutput=False)
    bv_d = nc.declare_dram_parameter("b_v", [1, Cp], bf, isOutput=False)
    bp_d = nc.declare_dram_parameter("b_proj", [1, Cp], bf, isOutput=False)
    tril_d = nc.declare_dram_parameter("tril", [P, 2 * P], bf, isOutput=False)
    out_d = nc.declare_dram_parameter("out", [Tp, Cp], bf, isOutput=True)

    with tile.TileContext(nc) as tc, ExitStack() as ctx:
        consts = ctx.enter_context(tc.tile_pool(name="consts", bufs=1))
        epool = ctx.enter_context(tc.tile_pool(name="epool", bufs=2))
        rpool = ctx.enter_context(tc.tile_pool(name="rpool", bufs=2))
        psmm = ctx.enter_context(tc.tile_pool(name="psmm", bufs=2, space="PSUM"))
        pss = ctx.enter_context(tc.tile_pool(name="pss", bufs=2, space="PSUM"))
        psav = ctx.enter_context(tc.tile_pool(name="psav", bufs=1, space="PSUM"))

        # ---- persistent SBUF ----
        xT = consts.tile([P, CT, Tp], bf)
        wqk = consts.tile([P, CT, 2 * Cp], bf)
        wv = consts.tile([P, CT, Cp], bf)
        wp = consts.tile([P, CT, Cp], bf)
        qkT = consts.tile([P, NU, Tp], bf)
        # zero-padded per-head k: head 2u in partitions 0:64 of slot 2u,
        # head 2u+1 in partitions 64:128 of slot 2u+1; the complementary
        # halves stay zero so S matmuls contract over the full 128
        # partitions (bf16 double-pump needs 128-row contraction).
        kpad = consts.tile([P, Hp, Tp], bf)
        Vp = consts.tile([P, TT, Hp, HDp + 1], bf)
        YT = consts.tile([P, CT, Tp], bf)
        tril = consts.tile([P, 2, P], bf)
        ones = consts.tile([1, P], bf)
        bqk = consts.tile([P, M2C], f32) if with_bias_qk else None
        bv = consts.tile([1, Cp], bf) if with_bias_v else None
        bp = consts.tile([1, Cp], bf) if with_bias_p else None

        # ---- DMAs: small consts, then chunks in compute order ----
        nc.sync.dma_start(tril[:], tril_d.rearrange("p (j q) -> p j q", j=2))
        if with_bias_qk:
            nc.sync.dma_start(bqk[:], bqk_d.rearrange("m p -> p m"))
        if with_bias_v:
            nc.sync.dma_start(bv[:], bv_d[:])

        def dma_xt(tj, ct_groups=1):
            tsl = slice(tj * TCH, (tj + 1) * TCH)
            cg = CT // ct_groups
            for g in range(ct_groups):
                csl = slice(g * cg, (g + 1) * cg)
                nc.sync.dma_start(
                    xT[:, csl, tsl],
                    xT_d[g * cg * P:(g + 1) * cg * P, tsl].rearrange(
                        "(ct p) t -> p ct t", p=P),
                )

        def dma_wqk(m, eng=None):
            eng = eng or nc.sync
            msl = slice(m * P, (m + 1) * P)
            eng.dma_start(
                wqk[:, :, msl],
                wqk_d[:, msl].rearrange("(ct p) n -> p ct n", p=P),
            )

        def dma_wv(vj, eng=None):
            eng = eng or nc.sync
            vsl = slice(vj * QC, (vj + 1) * QC)
            eng.dma_start(
                wv[:, :, vsl],
                wv_d[:, vsl].rearrange("(ct p) n -> p ct n", p=P),
            )

        def dma_wp(zj, eng=None):
            eng = eng or nc.sync
            zsl = slice(zj * QC, (zj + 1) * QC)
            eng.dma_start(
                wp[:, :, zsl],
                wp_d[:, zsl].rearrange("(ct p) n -> p ct n", p=P),
            )

        cg = CT // 2
        # split the input load across both hwdge queues (SP + ACT):
        # xT halves on SP while wqk m0/m8 stream on ACT in parallel
        nc.sync.dma_start(
            xT[:, 0:cg, 0:TCH],
            xT_d[0:cg * P, 0:TCH].rearrange("(ct p) t -> p ct t", p=P),
        )
        dma_wqk(0, eng=nc.scalar)
        nc.sync.dma_start(
            xT[:, cg:CT, 0:TCH],
            xT_d[cg * P:CT * P, 0:TCH].rearrange("(ct p) t -> p ct t", p=P),
        )
        dma_wqk(M2C // 2, eng=nc.scalar)
        dma_wv(0)
        for u in range(1, NU):
            dma_wqk(u, eng=nc.scalar)
            dma_wqk(M2C // 2 + u)
            if u == 2:
                dma_wv(VJ - 1, eng=nc.scalar)
            if u == 3 and TJ > 1:
                dma_xt(1, ct_groups=1)
        for m in range(NU, M2C // 2):
            dma_wqk(m, eng=nc.scalar)
            dma_wqk(M2C // 2 + m)
        for zj in range(VJ):
            dma_wp(zj, eng=(nc.scalar if zj % 2 else nc.sync))
        if with_bias_p:
            nc.sync.dma_start(bp[:], bp_d[:])

        nc.gpsimd.memset(ones[:], 1.0)
        # only the augmented-ones column of Vp needs init
        nc.gpsimd.memset(Vp[:, :, :, HDp:HDp + 1], 1.0)
        # zero halves of kpad (written once; data halves refresh per body)
        for h in range(Hp):
            if h % 2 == 0:
                nc.gpsimd.memset(kpad[HDp:P, h, :], 0.0)
            else:
                nc.gpsimd.memset(kpad[0:HDp, h, :], 0.0)

        # ---- GEMM group emitters ----
        def emit_qk(m, tj):
            msl = slice(m * P, (m + 1) * P)
            tsl = slice(tj * TCH, (tj + 1) * TCH)
            ps = psmm.tile([P, TCH], f32, tag="mm")
            for ct in range(CT):
                nc.tensor.matmul(
                    ps[:], lhsT=wqk[:, ct, msl], rhs=xT[:, ct, tsl],
                    start=(ct == 0), stop=(ct == CT - 1),
                )
            if m < M2C // 2:
                if with_bias_qk:
                    nc.vector.tensor_scalar_add(
                        qkT[:, m, tsl], ps[:], bqk[:, m:m + 1],
                    )
                else:
                    nc.vector.tensor_copy(out=qkT[:, m, tsl], in_=ps[:])
            else:
                # k pair chunk -> split halves into zero-padded head slots
                u2 = m - M2C // 2
                if with_bias_qk:
                    nc.vector.tensor_scalar_add(
                        kpad[0:HDp, 2 * u2, tsl], ps[0:HDp, :],
                        bqk[0:HDp, m:m + 1],
                    )
                    nc.vector.tensor_scalar_add(
                        kpad[HDp:P, 2 * u2 + 1, tsl], ps[HDp:P, :],
                        bqk[HDp:P, m:m + 1],
                    )
                else:
                    nc.vector.tensor_copy(
                        out=kpad[0:HDp, 2 * u2, tsl], in_=ps[0:HDp, :],
                    )
                    nc.scalar.activation(
                        kpad[HDp:P, 2 * u2 + 1, tsl], ps[HDp:P, :], AF.Copy,
                    )

        def emit_v(ti, vj):
            tsl = slice(ti * P, (ti + 1) * P)
            vsl = slice(vj * QC, (vj + 1) * QC)
            ps = psmm.tile([P, QC], f32, tag="mm")
            for ct in range(CT):
                nc.tensor.matmul(
                    ps[:], lhsT=xT[:, ct, tsl], rhs=wv[:, ct, vsl],
                    start=(ct == 0),
                    stop=(ct == CT - 1 and not with_bias_v),
                )
            if with_bias_v:
                nc.tensor.matmul(
                    ps[:], lhsT=ones[0:1, 0:P], rhs=bv[0:1, vsl],
                    start=False, stop=True,
                )
            hpc = QC // HDp
            nc.vector.tensor_copy(
                out=Vp[:, ti, vj * hpc:(vj + 1) * hpc, 0:HDp],
                in_=ps[:].rearrange("p (h d) -> p h d", d=HDp),
            )

        def emit_proj(ti, zj):
            tsl = slice(ti * P, (ti + 1) * P)
            zsl = slice(zj * QC, (zj + 1) * QC)
            ps = psmm.tile([P, QC], f32, tag="mm")
            for ct in range(CT):
                nc.tensor.matmul(
                    ps[:], lhsT=YT[:, ct, tsl], rhs=wp[:, ct, zsl],
                    start=(ct == 0),
                    stop=(ct == CT - 1 and not with_bias_p),
                )
            if with_bias_p:
                nc.tensor.matmul(
                    ps[:], lhsT=ones[0:1, 0:P], rhs=bp[0:1, zsl],
                    start=False, stop=True,
                )
            zt = rpool.tile([P, QC], bf, tag="zt")
            if ti >= TT // 2:
                # tail groups: ACT is idle after the last exp
                nc.scalar.activation(zt[:], ps[:], AF.Copy)
            else:
                nc.vector.tensor_copy(out=zt[:], in_=ps[:])
            nc.sync.dma_start(out_d[tsl, zsl], zt[:])

        # ---- work queue of GEMM groups ----
        queue = []
        queue += [("qk", 0, 0), ("qk", M2C // 2, 0)]
        queue += [("v", ti, 0) for ti in range(DIAG)]
        for u in range(1, NU):
            queue += [("qk", u, 0), ("qk", M2C // 2 + u, 0)]
            if u == 1:
                queue += [("v", 0, VJ - 1), ("v", 1, VJ - 1)]
            elif u == 2:
                queue += [("v", 2, VJ - 1), ("v", 3, VJ - 1)]
            elif u == 3:
                queue += [("v", 4, 0), ("v", 5, 0)]
            elif u == 4:
                queue += [("v", 6, 0), ("v", 7, 0)]
            elif u == 5:
                queue += [("v", 4, VJ - 1), ("v", 5, VJ - 1)]
            elif u == 6:
                queue += [("v", 6, VJ - 1), ("v", 7, VJ - 1)]
        if TJ > 1:
            for u in range(NU):
                queue += [("qk", u, 1), ("qk", M2C // 2 + u, 1)]
        for ti in range(TT):
            for zj in range(VJ):
                queue += [("proj", ti, zj)]

        emitted = set()
        proj_gate = [False, False]  # per qj row-half

        def emit_item(item):
            kind = item[0]
            if item in emitted:
                return
            emitted.add(item)
            if kind == "qk":
                emit_qk(item[1], item[2])
            elif kind == "v":
                emit_v(item[1], item[2])
            else:
                emit_proj(item[1], item[2])

        def ready(item):
            if item[0] != "proj":
                return True
            return proj_gate[item[1] // (TT // NQ)]

        def pump(n=1):
            done = 0
            i = 0
            while i < len(queue) and done < n:
                item = queue[i]
                if item in emitted:
                    queue.pop(i)
                    continue
                if ready(item):
                    queue.pop(i)
                    emit_item(item)
                    done += 1
                else:
                    i += 1

        def force(item):
            if item not in emitted:
                emit_item(item)

        # ---- attention ----
        def emit_attn_unit(u, qj, deferred):
            force(("qk", u, 0))
            force(("qk", M2C // 2 + u, 0))
            if qj > 0:
                force(("qk", u, 1))
                force(("qk", M2C // 2 + u, 1))
            nk = DIAG * (qj + 1)
            q0 = qj * QC
            vj_u = (2 * u) * HDp // QC  # head column chunk for this unit

            E = epool.tile([P, DIAG * NQ, 2, QC], fp8e3, tag="E")
            # S + exp per k-tile (both heads in one PSUM tile / one exp);
            # full-128 contraction: kpad zero-halves mask the other head's
            # q rows in the shared rhs.
            for ki in range(nk):
                off = max(0, P * (ki - DIAG * qj))
                ksl = slice(ki * P, (ki + 1) * P)
                ps = pss.tile([P, 2, QC], f32, tag="ss",
                              name=f"ss_{u}_{qj}_{ki}")
                for hh in range(2):
                    nc.tensor.matmul(
                        ps[:, hh, off:],
                        lhsT=kpad[:, 2 * u + hh, ksl],
                        rhs=qkT[:, u, q0 + off:q0 + QC],
                        start=True, stop=True,
                    )
                nc.scalar.activation(E[:, ki, :, off:], ps[:, :, off:], AF.Exp)
                if ki % 2 == 1:
                    if deferred and ki == 1:
                        deferred()
                        deferred = None
                    else:
                        pump(1)
            if deferred:
                deferred()
                deferred = None
            # causal mask on diagonal blocks (Pool engine)
            for rel in range(DIAG):
                ki = DIAG * qj + rel
                off = P * rel
                blk = slice(off, off + P)
                nc.vector.tensor_mul(
                    out=E[:, ki, :, blk], in0=E[:, ki, :, blk],
                    in1=tril[:],
                )
            # force V tiles needed by AV
            for ti in range(nk):
                force(("v", ti, vj_u))
            # AV chains (sequential per head; full 128-row contraction)
            pav = psav.tile([P, 2, QC], f32, tag="av", name=f"av_{u}_{qj}")
            for hh, h in ((0, 2 * u), (1, 2 * u + 1)):
                for ki in range(nk):
                    off = max(0, P * (ki - DIAG * qj))
                    nc.tensor.matmul(
                        pav[0:HDp + 1, hh, off:],
                        lhsT=Vp[:, ki, h, :], rhs=E[:, ki, hh, off:],
                        start=(ki == 0), stop=(ki == nk - 1),
                    )
            rrA = rpool.tile([1, QC], bf, tag="rrA")
            rrB = rpool.tile([1, QC], bf, tag="rrB")
            with nc.allow_low_precision(
                reason="bf16 softmax-recip feeds bf16 scale"
            ):
                nc.vector.reciprocal(rrA[:], pav[HDp:HDp + 1, 0, :])
                nc.vector.reciprocal(rrB[:], pav[HDp:HDp + 1, 1, :])

            qsl = slice(q0, q0 + QC)

            def epilogue():
                if USE_PB:
                    rbA = rpool.tile([HDp, QC], bf, tag="rbA")
                    rbB = rpool.tile([HDp, QC], bf, tag="rbB")
                    nc.gpsimd.partition_broadcast(rbA[:], rrA[:], channels=HDp)
                    nc.gpsimd.partition_broadcast(rbB[:], rrB[:], channels=HDp)
                    in1A, in1B = rbA[:], rbB[:]
                else:
                    nc.tensor.matmul(
                        pav[HDp:P, 0, :], lhsT=ones[0:1, 0:HDp], rhs=rrA[:],
                        start=True, stop=True,
                    )
                    nc.tensor.matmul(
                        pav[HDp:P, 1, :], lhsT=ones[0:1, 0:HDp], rhs=rrB[:],
                        start=True, stop=True,
                    )
                    # HW allows only one PSUM operand per TensorTensor:
                    # stage the broadcast rows through SBUF
                    rb = rpool.tile([HDp, 2, QC], bf, tag="rb")
                    nc.vector.tensor_copy(out=rb[:], in_=pav[HDp:P, :, :])
                    in1A, in1B = rb[:, 0, :], rb[:, 1, :]
                nc.vector.tensor_mul(
                    out=YT[0:HDp, u, qsl], in0=pav[0:HDp, 0, :], in1=in1A,
                )
                nc.vector.tensor_mul(
                    out=YT[HDp:P, u, qsl], in0=pav[0:HDp, 1, :], in1=in1B,
                )

            return epilogue

        def body():
            deferred = None
            for qj in range(NQ):
                for u in range(NU):
                    deferred = emit_attn_unit(u, qj, deferred)
                # last unit of this qj: emit epilogue now so proj can flow
                deferred()
                deferred = None
                proj_gate[qj] = True
                if qj == 0:
                    pump(2)
            # drain remaining GEMM groups (proj of second row-half etc.)
            pump(len(queue) + 1)

        if reps == 1:
            body()
        else:
            hint = (
                mybir.EngineType.PE,
                mybir.EngineType.DVE,
                mybir.EngineType.Activation,
                mybir.EngineType.Pool,
            )
            with tc.For_i(0, reps, 1, hint_engines=hint):
                body()

    nc.finalize()
    return nc


def _prep_shared(w_attn, b_attn, w_proj, b_proj):
    """Host-side layout marshalling of the replicated weights (bf16 cast,
    per-head q/k/v column gather, exact 1/8 q pre-scale)."""
    wr = np.asarray(w_attn, np.float32).reshape(C, H, 3, HD)
    w_q = (wr[:, :, 0, :] * np.float32(0.125)).reshape(C, C)
    w_k = wr[:, :, 1, :].reshape(C, C)
    w_qk = np.ascontiguousarray(
        np.concatenate([w_q, w_k], axis=1)
    ).astype(BF16)
    w_v = np.ascontiguousarray(wr[:, :, 2, :].reshape(C, C)).astype(BF16)

    br = np.asarray(b_attn, np.float32).reshape(H, 3, HD)
    b_qk = np.ascontiguousarray(
        np.concatenate(
            [(br[:, 0, :] * np.float32(0.125)).reshape(C), br[:, 1, :].reshape(C)]
        ).reshape(2 * C // 128, 128)
    )
    b_v = np.ascontiguousarray(br[:, 2, :].reshape(1, C)).astype(BF16)

    wp = np.ascontiguousarray(np.asarray(w_proj, np.float32)).astype(BF16)
    bp = np.ascontiguousarray(
        np.asarray(b_proj, np.float32).reshape(1, C)
    ).astype(BF16)

    k_idx = np.arange(128)[:, None]
    q_idx = np.arange(128)[None, :]
    t1 = (k_idx <= q_idx).astype(BF16)
    tril = np.concatenate([t1, t1], axis=1)
    return w_qk, w_v, wp, b_qk, b_v, bp, tril


class _Runner:
    """Cached jit(shard_map) executor for a prebuilt Bass module across
    N cores — same lowering as bass2jax.run_bass_via_pjrt, but reusable
    across calls so warm executions can be timed."""

    def __init__(self, nc, n_cores):
        import jax
        import numpy as _np
        from jax.sharding import Mesh, PartitionSpec
        try:
            from jax.experimental.shard_map import shard_map
        except ImportError:
            from jax.shard_map import shard_map
        from concourse import bass2jax, mybir

        bass2jax.install_neuronx_cc_hook()
        assert not nc.dbg_callbacks
        self.dbg_name = nc.dbg_addr.name if nc.dbg_addr is not None else None
        partition_name = (
            nc.partition_id_tensor.name if nc.partition_id_tensor else None
        )

        in_names, out_names, out_avals = [], [], []
        for alloc in nc.m.functions[0].allocations:
            if not isinstance(alloc, mybir.MemoryLocationSet):
                continue
            name = alloc.memorylocations[0].name
            if alloc.kind == "ExternalInput":
                if name != partition_name:
                    in_names.append(name)
            elif alloc.kind == "ExternalOutput":
                out_names.append(name)
                out_avals.append(
                    jax.core.ShapedArray(
                        tuple(alloc.tensor_shape), mybir.dt.np(alloc.dtype)
                    )
                )
        self.n_params = len(in_names)
        self.in_names = list(in_names)
        self.out_names = out_names
        self.out_avals = out_avals
        self.n_cores = n_cores
        all_names = in_names + out_names
        if partition_name is not None:
            all_names = all_names + [partition_name]

        def _body(*args):
            operands = list(args)
            if partition_name is not None:
                operands.append(bass2jax.partition_id_tensor())
            outs = bass2jax._bass_exec_p.bind(
                *operands,
                out_avals=tuple(out_avals),
                in_names=tuple(all_names),
                out_names=tuple(out_names),
                lowering_input_output_aliases=(),
                sim_require_finite=True,
                sim_require_nnan=True,
                nc=nc,
            )
            return tuple(outs)

        devices = jax.devices()[:n_cores]
        mesh = Mesh(_np.asarray(devices), ("core",))
        n_outs = len(out_names)
        self.jitted = jax.jit(
            shard_map(
                _body,
                mesh=mesh,
                in_specs=(PartitionSpec("core"),) * (self.n_params + n_outs),
                out_specs=(PartitionSpec("core"),) * n_outs,
                check_rep=False,
            ),
            keep_unused=True,
        )
        from jax.sharding import NamedSharding

        self.sharding = NamedSharding(mesh, PartitionSpec("core"))
        self.dev_zeros = [
            jax.device_put(
                _np.zeros((n_cores * a.shape[0], *a.shape[1:]), a.dtype),
                self.sharding,
            )
            for a in out_avals
        ]

    def prep_args(self, in_maps):
        import jax
        import numpy as _np

        if self.dbg_name is not None:
            dbg = _np.zeros((1, 2), _np.uint32)
            in_maps = [{**m, self.dbg_name: dbg} for m in in_maps]
        return [
            jax.device_put(
                _np.concatenate(
                    [_np.asarray(m[name]) for m in in_maps], axis=0
                ),
                self.sharding,
            )
            for name in self.in_names
        ]

    def run(self, concat_in):
        import jax

        out = self.jitted(*concat_in, *self.dev_zeros)
        return jax.block_until_ready(out)

    def results(self, out_arrs):
        import numpy as _np

        return [
            {
                name: _np.asarray(out_arrs[i]).reshape(
                    self.n_cores, *self.out_avals[i].shape
                )[c]
                for i, name in enumerate(self.out_names)
            }
            for c in range(self.n_cores)
        ]


_RUNNER_CACHE = {}


def _get_runner(reps=1, with_bias_qk=False, with_bias_v=True, with_bias_p=True):
    key = (T, C, H, HD, reps, with_bias_qk, with_bias_v, with_bias_p)
    if key not in _RUNNER_CACHE:
        if key not in _NC_CACHE:
            _NC_CACHE[key] = _build_nc(
                T, C, H, HD, reps=reps, with_bias_qk=with_bias_qk,
                with_bias_v=with_bias_v, with_bias_p=with_bias_p,
            )
        _RUNNER_CACHE[key] = _Runner(_NC_CACHE[key], N_CORES)
    return _RUNNER_CACHE[key]


def _make_in_maps(x, w_attn, b_attn, w_proj, b_proj):
    w_qk, w_v, wp, b_qk, b_v, bp, tril = _prep_shared(
        w_attn, b_attn, w_proj, b_proj
    )
    x = np.asarray(x, np.float32)
    in_maps = []
    for i in range(N_CORES):
        xT = np.ascontiguousarray(x[i].T).astype(BF16)
        in_maps.append({
            "xT": xT, "w_qk": w_qk, "w_v": w_v, "w_proj": wp,
            "b_qk": b_qk, "b_v": b_v, "b_proj": bp, "tril": tril,
        })
    return in_maps


def _bias_flags(b_attn, b_proj):
    br = np.asarray(b_attn, np.float32).reshape(H, 3, HD)
    bqk = bool(br[:, 0, :].any() or br[:, 1, :].any())
    bv = bool(br[:, 2, :].any())
    return bqk, bv, bool(np.asarray(b_proj, np.float32).any())


def kernel(x, w_attn, b_attn, w_proj, b_proj):
    wbqk, wbv, wbp = _bias_flags(b_attn, b_proj)
    runner = _get_runner(with_bias_qk=wbqk, with_bias_v=wbv, with_bias_p=wbp)
    concat_in = runner.prep_args(
        _make_in_maps(x, w_attn, b_attn, w_proj, b_proj)
    )
    res = runner.results(runner.run(concat_in))
    return np.stack([res[i]["out"] for i in range(N_CORES)]).astype(np.float32)


def measure(x, w_attn, b_attn, w_proj, b_proj, iters=5, reps=1):
    """Warm wall-clock times (s) of the sharded on-device execution."""
    import time

    wbqk, wbv, wbp = _bias_flags(b_attn, b_proj)
    runner = _get_runner(reps=reps, with_bias_qk=wbqk, with_bias_v=wbv,
                         with_bias_p=wbp)
    concat_in = runner.prep_args(
        _make_in_maps(x, w_attn, b_attn, w_proj, b_proj)
    )
    runner.run(concat_in)  # warm-up / compile
    times = []
    for _ in range(iters):
        t0 = time.perf_counter()
        runner.run(concat_in)
        times.append(time.perf_counter() - t0)
    return times

